# revision 2
# baseline (speedup 1.0000x reference)
"""Attention2D Trainium2 Bass kernel.

Reference computation (per sample s of 4):
    x  = GroupNorm32(q[s])                      # [512, 4096] (c, hw)
    qp = Wq xn + bq ; kp = Wk xn + bk ; vp = Wv xn + bv
    S[i, j]  = sum_c kp[c, i] qp[c, j] / sqrt(512)
    A[:, j]  = softmax_i(S[:, j])
    out[c,j] = sum_i vp[c, i] A[i, j]
    y        = (Wo out + bo + q[s]) / sqrt(2)

Sharding: 8 cores = 4 samples x 2 query-halves (2048 tokens each).
The host permutes the token axis per core so the core's query half is
always tokens [0:2048) -> every core runs an identical program (SPMD,
no collectives).  Key/value work over all 4096 tokens is duplicated
between the two cores of a sample (cheap relative to attention).

On-chip layout: scores are computed as S[i(keys on partitions), j] so
that exp() is a single ScalarE pass PSUM->SBUF and the softmax
denominator Z[j] = sum_i E[i, j] is a ones-vector matmul on TensorE --
no transposes anywhere.  V is produced directly transposed (vfT[i, c])
by swapping matmul operands.  All matmul operands are bf16 (full PE
speed); accumulation in fp32 PSUM; GroupNorm stats in fp32.
"""

import numpy as np
import ml_dtypes

import concourse.bass as bass
import concourse.bacc as bacc
import concourse.tile as tile
import concourse.mybir as mybir
from concourse.bass_utils import run_bass_kernel_spmd

F32 = mybir.dt.float32
BF16 = mybir.dt.bfloat16
AF = mybir.ActivationFunctionType

P = 128          # partitions
C = 512          # channels
CT = C // P      # channel tiles (4)
T = 4096         # tokens per sample (h*w)
NQ = 2048        # query tokens per core
JC = 512         # query chunk (PSUM bank width in fp32)
NJ = NQ // JC    # query chunks per core (4)
IT = T // P      # key tiles (32)
NG_TILE = 8      # groups per channel tile (32 groups / 4 tiles)
GS = 16          # channels per group
EPS = 1e-6
SCALE = 1.0 / np.sqrt(C)
INV_SQRT2 = 0.7071067811865476


def build_bass():
    nc = bacc.Bacc("TRN2", target_bir_lowering=False, debug=False)

    x_d = nc.dram_tensor("x", [C, T], F32, kind="ExternalInput").ap()
    w_d = {
        n: nc.dram_tensor(n, [C, C], BF16, kind="ExternalInput").ap()
        for n in ("wqT", "wkT", "wvT", "woT")
    }
    # per-channel vectors in [p, tile] layout (c = t*128 + p)
    vec_d = {
        n: nc.dram_tensor(n, [P, CT], F32, kind="ExternalInput").ap()
        for n in ("bq", "bk", "bo2", "gamma", "beta")
    }
    bvrep_d = nc.dram_tensor("bv_rep", [P, C], F32, kind="ExternalInput").ap()
    indf_d = nc.dram_tensor("indf", [P, NG_TILE], F32, kind="ExternalInput").ap()
    indb_d = nc.dram_tensor("indb", [NG_TILE, P], F32, kind="ExternalInput").ap()
    onesb_d = nc.dram_tensor("ones_bf", [P, 1], BF16, kind="ExternalInput").ap()
    onesf_d = nc.dram_tensor("ones_f1", [1, P], F32, kind="ExternalInput").ap()
    y_d = nc.dram_tensor("y", [C, NQ], F32, kind="ExternalOutput").ap()

    with tile.TileContext(nc) as tc:
        with (
            tc.tile_pool(name="const", bufs=1) as const,
            tc.tile_pool(name="big", bufs=1) as big,
        ):
            # ---- constants ----
            w_sb = {}
            for n, d in w_d.items():
                w_sb[n] = const.tile([P, CT, C], BF16, tag=n, name=n)
                nc.sync.dma_start(out=w_sb[n], in_=d.rearrange("(t p) o -> p t o", p=P))
            vec_sb = {}
            for n, d in vec_d.items():
                vec_sb[n] = const.tile([P, CT], F32, tag=n, name=n)
                nc.sync.dma_start(out=vec_sb[n], in_=d)
            bvrep = const.tile([P, C], F32, tag="bvrep")
            nc.sync.dma_start(out=bvrep, in_=bvrep_d)
            indf = const.tile([P, NG_TILE], F32, tag="indf")
            nc.sync.dma_start(out=indf, in_=indf_d)
            indb = const.tile([NG_TILE, P], F32, tag="indb")
            nc.sync.dma_start(out=indb, in_=indb_d)
            ones_bf = const.tile([P, 1], BF16, tag="onesb")
            nc.sync.dma_start(out=ones_bf, in_=onesb_d)
            ones_f1 = const.tile([1, P], F32, tag="onesf")
            nc.sync.dma_start(out=ones_f1, in_=onesf_d)
            eps_t = const.tile([P, 1], F32, tag="eps")
            nc.vector.memset(eps_t, EPS)

            # ---- persistent activations ----
            xn = big.tile([P, CT, T], BF16, tag="xn")     # normalized input
            kf = big.tile([P, CT, T], BF16, tag="kf")     # K  [c, i]
            qf = big.tile([P, CT, NQ], BF16, tag="qf")    # Q  [c, j]
            vfT = big.tile([P, IT, C], BF16, tag="vfT")   # V^T [i, c]

            # ================= phase 1: GroupNorm =================
            with (
                tc.tile_pool(name="sbx", bufs=2) as sbx,
                tc.tile_pool(name="st", bufs=2) as st,
                tc.tile_pool(name="psg", bufs=2, space="PSUM") as psg,
            ):
                for t in range(CT):
                    x_t = sbx.tile([P, T], F32, tag="x")
                    nc.sync.dma_start(out=x_t, in_=x_d[t * P:(t + 1) * P, :])
                    stats = st.tile([P, 8, 6], F32, tag="stats")
                    for sg in range(8):
                        nc.vector.bn_stats(
                            out=stats[:, sg, :], in_=x_t[:, sg * 512:(sg + 1) * 512]
                        )
                    mv = st.tile([P, 2], F32, tag="mv")
                    nc.vector.bn_aggr(out=mv, in_=stats)
                    # t2 = [mean, E[x^2]] per channel
                    t2 = st.tile([P, 2], F32, tag="t2")
                    nc.vector.tensor_copy(out=t2[:, 0:1], in_=mv[:, 0:1])
                    nc.vector.tensor_mul(t2[:, 1:2], mv[:, 0:1], mv[:, 0:1])
                    nc.vector.tensor_add(t2[:, 1:2], t2[:, 1:2], mv[:, 1:2])
                    # group-reduce (mean over the 16 channels of each group)
                    g_ps = psg.tile([NG_TILE, 2], F32, tag="g")
                    nc.tensor.matmul(g_ps, indf, t2, start=True, stop=True)
                    gm = st.tile([NG_TILE, 2], F32, tag="gm")
                    nc.vector.tensor_copy(out=gm, in_=g_ps)
                    var = st.tile([NG_TILE, 1], F32, tag="var")
                    nc.vector.tensor_mul(var, gm[:, 0:1], gm[:, 0:1])
                    nc.vector.tensor_sub(var, gm[:, 1:2], var)
                    sd = st.tile([NG_TILE, 1], F32, tag="sd")
                    nc.scalar.activation(
                        out=sd, in_=var, func=AF.Sqrt, bias=eps_t[0:NG_TILE, :]
                    )
                    rstd = st.tile([NG_TILE, 1], F32, tag="rstd")
                    nc.vector.reciprocal(out=rstd, in_=sd)
                    gm2 = st.tile([NG_TILE, 2], F32, tag="gm2")
                    nc.vector.tensor_copy(out=gm2[:, 0:1], in_=gm[:, 0:1])
                    nc.vector.tensor_copy(out=gm2[:, 1:2], in_=rstd)
                    # broadcast group stats back to channels
                    bc_ps = psg.tile([P, 2], F32, tag="g")
                    nc.tensor.matmul(bc_ps, indb, gm2, start=True, stop=True)
                    a_t = st.tile([P, 1], F32, tag="a")
                    b_t = st.tile([P, 1], F32, tag="b")
                    nc.vector.tensor_mul(a_t, bc_ps[:, 1:2], vec_sb["gamma"][:, t:t + 1])
                    nc.vector.tensor_mul(b_t, bc_ps[:, 0:1], a_t)
                    nc.vector.tensor_sub(b_t, vec_sb["beta"][:, t:t + 1], b_t)
                    nc.scalar.activation(
                        out=xn[:, t, :], in_=x_t, func=AF.Identity,
                        bias=b_t, scale=a_t,
                    )

            # ================= phase 2: Q/K/V projections =================
            with tc.tile_pool(name="psp", bufs=3, space="PSUM") as psp:
                # K: kf[c_out, i] ; Q: qf[c_out, j] (queries are tokens 0:NQ)
                for t_out in range(CT):
                    for ic in range(T // JC):
                        kp = psp.tile([P, JC], F32, tag="p")
                        for t in range(CT):
                            nc.tensor.matmul(
                                kp,
                                w_sb["wkT"][:, t, t_out * P:(t_out + 1) * P],
                                xn[:, t, ic * JC:(ic + 1) * JC],
                                start=(t == 0), stop=(t == CT - 1),
                            )
                        nc.scalar.activation(
                            out=kf[:, t_out, ic * JC:(ic + 1) * JC], in_=kp,
                            func=AF.Identity, bias=vec_sb["bk"][:, t_out:t_out + 1],
                        )
                for t_out in range(CT):
                    for jc in range(NJ):
                        qp = psp.tile([P, JC], F32, tag="p")
                        for t in range(CT):
                            nc.tensor.matmul(
                                qp,
                                w_sb["wqT"][:, t, t_out * P:(t_out + 1) * P],
                                xn[:, t, jc * JC:(jc + 1) * JC],
                                start=(t == 0), stop=(t == CT - 1),
                            )
                        nc.scalar.activation(
                            out=qf[:, t_out, jc * JC:(jc + 1) * JC], in_=qp,
                            func=AF.Identity, bias=vec_sb["bq"][:, t_out:t_out + 1],
                        )
                # V, produced transposed: vfT[i, c] = sum_c' xn[c', i] wvT[c', c]
                for k in range(IT):
                    vp = psp.tile([P, C], F32, tag="p")
                    for t in range(CT):
                        nc.tensor.matmul(
                            vp,
                            xn[:, t, k * P:(k + 1) * P],
                            w_sb["wvT"][:, t, :],
                            start=(t == 0), stop=(t == CT - 1),
                        )
                    nc.vector.tensor_add(vfT[:, k, :], vp, bvrep)

            # ================= phase 3: attention =================
            with (
                tc.tile_pool(name="pss", bufs=2, space="PSUM") as pss,
                tc.tile_pool(name="psav", bufs=4, space="PSUM") as psav,
                tc.tile_pool(name="psz", bufs=2, space="PSUM") as psz,
                tc.tile_pool(name="sbe", bufs=3) as sbe,
                tc.tile_pool(name="sbw", bufs=2) as sbw,
                tc.tile_pool(name="sbq", bufs=8) as sbq,
                tc.tile_pool(name="sby", bufs=4) as sby,
            ):
                for jc in range(NJ):
                    # residual input for this chunk (original x, queries 0:NQ)
                    xqs = []
                    for mo in range(CT):
                        xq_t = sbq.tile([P, JC], F32, tag="xq")
                        nc.sync.dma_start(
                            out=xq_t,
                            in_=x_d[mo * P:(mo + 1) * P, jc * JC:(jc + 1) * JC],
                        )
                        xs = sbq.tile([P, JC], F32, tag="xqs")
                        nc.scalar.activation(
                            out=xs, in_=xq_t, func=AF.Identity,
                            bias=vec_sb["bo2"][:, mo:mo + 1], scale=INV_SQRT2,
                        )
                        xqs.append(xs)

                    z_ps = psz.tile([1, JC], F32, tag="z")
                    av_ps = [psav.tile([P, JC], F32, tag="av", name=f"av{m}") for m in range(CT)]
                    for k in range(IT):
                        s_ps = pss.tile([P, JC], F32, tag="s")
                        for t in range(CT):
                            nc.tensor.matmul(
                                s_ps,
                                kf[:, t, k * P:(k + 1) * P],
                                qf[:, t, jc * JC:(jc + 1) * JC],
                                start=(t == 0), stop=(t == CT - 1),
                            )
                        e_t = sbe.tile([P, JC], BF16, tag="e")
                        nc.scalar.activation(out=e_t, in_=s_ps, func=AF.Exp, scale=SCALE)
                        nc.tensor.matmul(
                            z_ps, ones_bf, e_t, start=(k == 0), stop=(k == IT - 1)
                        )
                        for m in range(CT):
                            nc.tensor.matmul(
                                av_ps[m],
                                vfT[:, k, m * P:(m + 1) * P],
                                e_t,
                                start=(k == 0), stop=(k == IT - 1),
                            )
                    # normalize: out_n = av / Z
                    zinv = sbw.tile([1, JC], F32, tag="zinv")
                    nc.vector.reciprocal(out=zinv, in_=z_ps)
                    zbc_ps = psz.tile([P, JC], F32, tag="z")
                    nc.tensor.matmul(zbc_ps, ones_f1, zinv, start=True, stop=True)
                    zrep = sbw.tile([P, JC], F32, tag="zrep")
                    nc.vector.tensor_copy(out=zrep, in_=zbc_ps)
                    out_n = sbw.tile([P, CT, JC], BF16, tag="outn")
                    for m in range(CT):
                        nc.vector.tensor_mul(out_n[:, m, :], av_ps[m], zrep)
                    # output projection + residual
                    for mo in range(CT):
                        y_ps = psav.tile([P, JC], F32, tag="av")
                        for m in range(CT):
                            nc.tensor.matmul(
                                y_ps,
                                w_sb["woT"][:, m, mo * P:(mo + 1) * P],
                                out_n[:, m, :],
                                start=(m == 0), stop=(m == CT - 1),
                            )
                        yf = sby.tile([P, JC], F32, tag="y")
                        nc.vector.scalar_tensor_tensor(
                            out=yf, in0=y_ps, scalar=INV_SQRT2, in1=xqs[mo],
                            op0=mybir.AluOpType.mult, op1=mybir.AluOpType.add,
                        )
                        nc.sync.dma_start(
                            out=y_d[mo * P:(mo + 1) * P, jc * JC:(jc + 1) * JC],
                            in_=yf,
                        )
    nc.compile()
    return nc


def make_in_maps(q, gamma, beta, wq, bq, wk, bk, wv, bv, wo, bo):
    """Host-side prep: per-core permuted x + replicated (pre-transposed) weights."""
    f32 = np.float32
    bf16 = ml_dtypes.bfloat16
    q = np.asarray(q, f32)
    b = q.shape[0]
    x = q.reshape(b, C, T)

    def pt(v):  # [512] -> [128, 4] (c = t*128 + p)
        return np.ascontiguousarray(np.asarray(v, f32).reshape(CT, P).T)

    common = {
        "wqT": np.ascontiguousarray(np.asarray(wq, f32).T).astype(bf16),
        "wkT": np.ascontiguousarray(np.asarray(wk, f32).T).astype(bf16),
        "wvT": np.ascontiguousarray(np.asarray(wv, f32).T).astype(bf16),
        "woT": np.ascontiguousarray(np.asarray(wo, f32).T).astype(bf16),
        "bq": pt(bq), "bk": pt(bk), "bo2": pt(np.asarray(bo, f32) * INV_SQRT2),
        "gamma": pt(gamma), "beta": pt(beta),
        "bv_rep": np.ascontiguousarray(
            np.broadcast_to(np.asarray(bv, f32), (P, C))),
        "indf": np.ascontiguousarray(
            (np.arange(P)[:, None] // GS == np.arange(NG_TILE)[None, :])
            .astype(f32) / GS),
        "indb": np.ascontiguousarray(
            (np.arange(P)[None, :] // GS == np.arange(NG_TILE)[:, None])
            .astype(f32)),
        "ones_bf": np.ones((P, 1), bf16),
        "ones_f1": np.ones((1, P), f32),
    }
    in_maps = []
    for core in range(8):
        s, half = divmod(core, 2)
        xs = x[s]
        if half == 0:
            xp = xs
        else:
            xp = np.concatenate([xs[:, NQ:], xs[:, :NQ]], axis=1)
        in_maps.append({"x": np.ascontiguousarray(xp), **common})
    return in_maps


def assemble_output(results, b=4, h=64, w=64):
    out = np.empty((b, C, T), np.float32)
    for core in range(8):
        s, half = divmod(core, 2)
        out[s][:, half * NQ:(half + 1) * NQ] = results[core]["y"]
    return out.reshape(b, C, h, w)


_NC = None


def get_nc():
    global _NC
    if _NC is None:
        _NC = build_bass()
    return _NC


def kernel(**inputs):
    in_maps = make_in_maps(**inputs)
    nc = get_nc()
    res = run_bass_kernel_spmd(nc, in_maps, core_ids=list(range(8)))
    return assemble_output(res.results)


if __name__ == "__main__":
    nc = get_nc()
    print("built + compiled ok")


# revision 3
# speedup vs baseline: 23.3896x; 23.3896x over previous
"""Attention2D Trainium2 Bass kernel.

Reference computation (per sample s of 4):
    x  = GroupNorm32(q[s])                      # [512, 4096] (c, hw)
    qp = Wq xn + bq ; kp = Wk xn + bk ; vp = Wv xn + bv
    S[i, j]  = sum_c kp[c, i] qp[c, j] / sqrt(512)
    A[:, j]  = softmax_i(S[:, j])
    out[c,j] = sum_i vp[c, i] A[i, j]
    y        = (Wo out + bo + q[s]) / sqrt(2)

Sharding: 8 cores = 4 samples x 2 query-halves (2048 tokens each).
The host permutes the token axis per core so the core's query half is
always tokens [0:2048) -> every core runs an identical program (SPMD,
no collectives).  Key/value work over all 4096 tokens is duplicated
between the two cores of a sample (cheap relative to attention).

On-chip layout: scores are computed as S[i(keys on partitions), j] so
that exp() is a single ScalarE pass PSUM->SBUF and the softmax
denominator Z[j] = sum_i E[i, j] is a ones-vector matmul on TensorE --
no transposes anywhere.  V is produced directly transposed (vfT[i, c])
by swapping matmul operands.  All matmul operands are bf16 (full PE
speed); accumulation in fp32 PSUM; GroupNorm stats in fp32.
"""

import numpy as np
import ml_dtypes

import concourse.bass as bass
import concourse.bacc as bacc
import concourse.tile as tile
import concourse.mybir as mybir
from concourse.bass_utils import run_bass_kernel_spmd

F32 = mybir.dt.float32
BF16 = mybir.dt.bfloat16
AF = mybir.ActivationFunctionType

P = 128          # partitions
C = 512          # channels
CT = C // P      # channel tiles (4)
T = 4096         # tokens per sample (h*w)
NQ = 2048        # query tokens per core
JC = 512         # query chunk (PSUM bank width in fp32)
NJ = NQ // JC    # query chunks per core (4)
IT = T // P      # key tiles (32)
NG_TILE = 8      # groups per channel tile (32 groups / 4 tiles)
GS = 16          # channels per group
EPS = 1e-6
SCALE = 1.0 / np.sqrt(C)
INV_SQRT2 = 0.7071067811865476


def _emit_body(nc, pools, aps):
    """One full forward pass. `pools` are long-lived tile pools; PSUM usage
    never exceeds 8 banks (pss 2 + psav 4 + psz 2)."""
    (const, big, sbx, st, sbe, sbw, sbq, sby, pss, psav, psz) = pools
    x_d, y_d, w_sb, vec_sb, bvrep, indf, indb, ones_bf, ones_f1, eps_t = aps

    # ---- persistent activations ----
    xn = big.tile([P, CT, T], BF16, tag="xn")     # normalized input
    kf = big.tile([P, CT, T], BF16, tag="kf")     # K  [c, i]
    qf = big.tile([P, CT, NQ], BF16, tag="qf")    # Q  [c, j]
    vfT = big.tile([P, IT, C], BF16, tag="vfT")   # V^T [i, c]

    # ================= phase 1: GroupNorm =================
    for t in range(CT):
        x_t = sbx.tile([P, T], F32, tag="x")
        nc.sync.dma_start(out=x_t, in_=x_d[t * P:(t + 1) * P, :])
        stats = st.tile([P, 8, 6], F32, tag="stats")
        for sg in range(8):
            nc.vector.bn_stats(
                out=stats[:, sg, :], in_=x_t[:, sg * 512:(sg + 1) * 512]
            )
        mv = st.tile([P, 2], F32, tag="mv")
        nc.vector.bn_aggr(out=mv, in_=stats)
        # t2 = [mean, E[x^2]] per channel
        t2 = st.tile([P, 2], F32, tag="t2")
        nc.vector.tensor_copy(out=t2[:, 0:1], in_=mv[:, 0:1])
        nc.vector.tensor_mul(t2[:, 1:2], mv[:, 0:1], mv[:, 0:1])
        nc.vector.tensor_add(t2[:, 1:2], t2[:, 1:2], mv[:, 1:2])
        # group-reduce (mean over the 16 channels of each group)
        g_ps = psz.tile([NG_TILE, 2], F32, tag="z")
        nc.tensor.matmul(g_ps, indf, t2, start=True, stop=True)
        gm = st.tile([NG_TILE, 2], F32, tag="gm")
        nc.vector.tensor_copy(out=gm, in_=g_ps)
        var = st.tile([NG_TILE, 1], F32, tag="var")
        nc.vector.tensor_mul(var, gm[:, 0:1], gm[:, 0:1])
        nc.vector.tensor_sub(var, gm[:, 1:2], var)
        sd = st.tile([NG_TILE, 1], F32, tag="sd")
        nc.scalar.activation(
            out=sd, in_=var, func=AF.Sqrt, bias=eps_t[0:NG_TILE, :]
        )
        rstd = st.tile([NG_TILE, 1], F32, tag="rstd")
        nc.vector.reciprocal(out=rstd, in_=sd)
        gm2 = st.tile([NG_TILE, 2], F32, tag="gm2")
        nc.vector.tensor_copy(out=gm2[:, 0:1], in_=gm[:, 0:1])
        nc.vector.tensor_copy(out=gm2[:, 1:2], in_=rstd)
        # broadcast group stats back to channels
        bc_ps = psz.tile([P, 2], F32, tag="z")
        nc.tensor.matmul(bc_ps, indb, gm2, start=True, stop=True)
        a_t = st.tile([P, 1], F32, tag="a")
        b_t = st.tile([P, 1], F32, tag="b")
        nc.vector.tensor_mul(a_t, bc_ps[:, 1:2], vec_sb["gamma"][:, t:t + 1])
        nc.vector.tensor_mul(b_t, bc_ps[:, 0:1], a_t)
        nc.vector.tensor_sub(b_t, vec_sb["beta"][:, t:t + 1], b_t)
        nc.scalar.activation(
            out=xn[:, t, :], in_=x_t, func=AF.Identity,
            bias=b_t, scale=a_t,
        )

    # ================= phase 2: Q/K/V projections =================
    # K: kf[c_out, i] ; Q: qf[c_out, j] (queries are tokens 0:NQ)
    for t_out in range(CT):
        for ic in range(T // JC):
            kp = pss.tile([P, JC], F32, tag="s")
            for t in range(CT):
                nc.tensor.matmul(
                    kp,
                    w_sb["wkT"][:, t, t_out * P:(t_out + 1) * P],
                    xn[:, t, ic * JC:(ic + 1) * JC],
                    start=(t == 0), stop=(t == CT - 1),
                )
            nc.scalar.activation(
                out=kf[:, t_out, ic * JC:(ic + 1) * JC], in_=kp,
                func=AF.Identity, bias=vec_sb["bk"][:, t_out:t_out + 1],
            )
    for t_out in range(CT):
        for jc in range(NJ):
            qp = pss.tile([P, JC], F32, tag="s")
            for t in range(CT):
                nc.tensor.matmul(
                    qp,
                    w_sb["wqT"][:, t, t_out * P:(t_out + 1) * P],
                    xn[:, t, jc * JC:(jc + 1) * JC],
                    start=(t == 0), stop=(t == CT - 1),
                )
            nc.scalar.activation(
                out=qf[:, t_out, jc * JC:(jc + 1) * JC], in_=qp,
                func=AF.Identity, bias=vec_sb["bq"][:, t_out:t_out + 1],
            )
    # V, produced transposed: vfT[i, c] = sum_c' xn[c', i] wvT[c', c]
    for k in range(IT):
        vp = pss.tile([P, C], F32, tag="s")
        for t in range(CT):
            nc.tensor.matmul(
                vp,
                xn[:, t, k * P:(k + 1) * P],
                w_sb["wvT"][:, t, :],
                start=(t == 0), stop=(t == CT - 1),
            )
        nc.vector.tensor_add(vfT[:, k, :], vp, bvrep)

    # ================= phase 3: attention =================
    for jc in range(NJ):
        # residual input for this chunk (original x, queries 0:NQ)
        xqs = []
        for mo in range(CT):
            xq_t = sbq.tile([P, JC], F32, tag="xq")
            nc.sync.dma_start(
                out=xq_t,
                in_=x_d[mo * P:(mo + 1) * P, jc * JC:(jc + 1) * JC],
            )
            xs = sbq.tile([P, JC], F32, tag="xqs")
            nc.scalar.activation(
                out=xs, in_=xq_t, func=AF.Identity,
                bias=vec_sb["bo2"][:, mo:mo + 1], scale=INV_SQRT2,
            )
            xqs.append(xs)

        z_ps = psz.tile([1, JC], F32, tag="z")
        av_ps = [psav.tile([P, JC], F32, tag="av", name=f"av{m}") for m in range(CT)]
        for k in range(IT):
            s_ps = pss.tile([P, JC], F32, tag="s")
            for t in range(CT):
                nc.tensor.matmul(
                    s_ps,
                    kf[:, t, k * P:(k + 1) * P],
                    qf[:, t, jc * JC:(jc + 1) * JC],
                    start=(t == 0), stop=(t == CT - 1),
                )
            e_t = sbe.tile([P, JC], BF16, tag="e")
            nc.scalar.activation(out=e_t, in_=s_ps, func=AF.Exp, scale=SCALE)
            nc.tensor.matmul(
                z_ps, ones_bf, e_t, start=(k == 0), stop=(k == IT - 1)
            )
            for m in range(CT):
                nc.tensor.matmul(
                    av_ps[m],
                    vfT[:, k, m * P:(m + 1) * P],
                    e_t,
                    start=(k == 0), stop=(k == IT - 1),
                )
        # normalize: out_n = av / Z
        zinv = sbw.tile([1, JC], F32, tag="zinv")
        nc.vector.reciprocal(out=zinv, in_=z_ps)
        zbc_ps = psz.tile([P, JC], F32, tag="z")
        nc.tensor.matmul(zbc_ps, ones_f1, zinv, start=True, stop=True)
        zrep = sbw.tile([P, JC], F32, tag="zrep")
        nc.vector.tensor_copy(out=zrep, in_=zbc_ps)
        out_n = sbw.tile([P, CT, JC], BF16, tag="outn")
        for m in range(CT):
            nc.vector.tensor_mul(out_n[:, m, :], av_ps[m], zrep)
        # output projection + residual
        for mo in range(CT):
            y_ps = psav.tile([P, JC], F32, tag="av")
            for m in range(CT):
                nc.tensor.matmul(
                    y_ps,
                    w_sb["woT"][:, m, mo * P:(mo + 1) * P],
                    out_n[:, m, :],
                    start=(m == 0), stop=(m == CT - 1),
                )
            yf = sby.tile([P, JC], F32, tag="y")
            nc.vector.scalar_tensor_tensor(
                out=yf, in0=y_ps, scalar=INV_SQRT2, in1=xqs[mo],
                op0=mybir.AluOpType.mult, op1=mybir.AluOpType.add,
            )
            nc.sync.dma_start(
                out=y_d[mo * P:(mo + 1) * P, jc * JC:(jc + 1) * JC],
                in_=yf,
            )


def build_bass(loop_iters=None):
    """loop_iters=None: single-shot kernel.  loop_iters=R: wrap the body in a
    hardware For_i loop (for on-device timing; everything re-runs each
    iteration, output is idempotent)."""
    nc = bacc.Bacc("TRN2", target_bir_lowering=False, debug=False)

    x_d = nc.dram_tensor("x", [C, T], F32, kind="ExternalInput").ap()
    w_d = {
        n: nc.dram_tensor(n, [C, C], BF16, kind="ExternalInput").ap()
        for n in ("wqT", "wkT", "wvT", "woT")
    }
    # per-channel vectors in [p, tile] layout (c = t*128 + p)
    vec_d = {
        n: nc.dram_tensor(n, [P, CT], F32, kind="ExternalInput").ap()
        for n in ("bq", "bk", "bo2", "gamma", "beta")
    }
    bvrep_d = nc.dram_tensor("bv_rep", [P, C], F32, kind="ExternalInput").ap()
    indf_d = nc.dram_tensor("indf", [P, NG_TILE], F32, kind="ExternalInput").ap()
    indb_d = nc.dram_tensor("indb", [NG_TILE, P], F32, kind="ExternalInput").ap()
    onesb_d = nc.dram_tensor("ones_bf", [P, 1], BF16, kind="ExternalInput").ap()
    onesf_d = nc.dram_tensor("ones_f1", [1, P], F32, kind="ExternalInput").ap()
    y_d = nc.dram_tensor("y", [C, NQ], F32, kind="ExternalOutput").ap()

    with tile.TileContext(nc) as tc:
        with (
            tc.tile_pool(name="const", bufs=1) as const,
            tc.tile_pool(name="big", bufs=1) as big,
            tc.tile_pool(name="sbx", bufs=2) as sbx,
            tc.tile_pool(name="st", bufs=2) as st,
            tc.tile_pool(name="sbe", bufs=3) as sbe,
            tc.tile_pool(name="sbw", bufs=2) as sbw,
            tc.tile_pool(name="sbq", bufs=4) as sbq,
            tc.tile_pool(name="sby", bufs=4) as sby,
            tc.tile_pool(name="pss", bufs=2, space="PSUM") as pss,
            tc.tile_pool(name="psav", bufs=4, space="PSUM") as psav,
            tc.tile_pool(name="psz", bufs=2, space="PSUM") as psz,
        ):
            # ---- constants (loaded once, outside any timing loop) ----
            w_sb = {}
            for n, d in w_d.items():
                w_sb[n] = const.tile([P, CT, C], BF16, tag=n, name=n)
                nc.sync.dma_start(out=w_sb[n], in_=d.rearrange("(t p) o -> p t o", p=P))
            vec_sb = {}
            for n, d in vec_d.items():
                vec_sb[n] = const.tile([P, CT], F32, tag=n, name=n)
                nc.sync.dma_start(out=vec_sb[n], in_=d)
            bvrep = const.tile([P, C], F32, tag="bvrep")
            nc.sync.dma_start(out=bvrep, in_=bvrep_d)
            indf = const.tile([P, NG_TILE], F32, tag="indf")
            nc.sync.dma_start(out=indf, in_=indf_d)
            indb = const.tile([NG_TILE, P], F32, tag="indb")
            nc.sync.dma_start(out=indb, in_=indb_d)
            ones_bf = const.tile([P, 1], BF16, tag="onesb")
            nc.sync.dma_start(out=ones_bf, in_=onesb_d)
            ones_f1 = const.tile([1, P], F32, tag="onesf")
            nc.sync.dma_start(out=ones_f1, in_=onesf_d)
            eps_t = const.tile([P, 1], F32, tag="eps")
            nc.vector.memset(eps_t, EPS)

            pools = (const, big, sbx, st, sbe, sbw, sbq, sby, pss, psav, psz)
            aps = (x_d, y_d, w_sb, vec_sb, bvrep, indf, indb, ones_bf,
                   ones_f1, eps_t)
            if loop_iters is None:
                _emit_body(nc, pools, aps)
            else:
                with tc.For_i(0, loop_iters, 1):
                    _emit_body(nc, pools, aps)
    nc.compile()
    return nc


def make_in_maps(q, gamma, beta, wq, bq, wk, bk, wv, bv, wo, bo):
    """Host-side prep: per-core permuted x + replicated (pre-transposed) weights."""
    f32 = np.float32
    bf16 = ml_dtypes.bfloat16
    q = np.asarray(q, f32)
    b = q.shape[0]
    x = q.reshape(b, C, T)

    def pt(v):  # [512] -> [128, 4] (c = t*128 + p)
        return np.ascontiguousarray(np.asarray(v, f32).reshape(CT, P).T)

    common = {
        "wqT": np.ascontiguousarray(np.asarray(wq, f32).T).astype(bf16),
        "wkT": np.ascontiguousarray(np.asarray(wk, f32).T).astype(bf16),
        "wvT": np.ascontiguousarray(np.asarray(wv, f32).T).astype(bf16),
        "woT": np.ascontiguousarray(np.asarray(wo, f32).T).astype(bf16),
        "bq": pt(bq), "bk": pt(bk), "bo2": pt(np.asarray(bo, f32) * INV_SQRT2),
        "gamma": pt(gamma), "beta": pt(beta),
        "bv_rep": np.ascontiguousarray(
            np.broadcast_to(np.asarray(bv, f32), (P, C))),
        "indf": np.ascontiguousarray(
            (np.arange(P)[:, None] // GS == np.arange(NG_TILE)[None, :])
            .astype(f32) / GS),
        "indb": np.ascontiguousarray(
            (np.arange(P)[None, :] // GS == np.arange(NG_TILE)[:, None])
            .astype(f32)),
        "ones_bf": np.ones((P, 1), bf16),
        "ones_f1": np.ones((1, P), f32),
    }
    in_maps = []
    for core in range(8):
        s, half = divmod(core, 2)
        xs = x[s]
        if half == 0:
            xp = xs
        else:
            xp = np.concatenate([xs[:, NQ:], xs[:, :NQ]], axis=1)
        in_maps.append({"x": np.ascontiguousarray(xp), **common})
    return in_maps


def assemble_output(results, b=4, h=64, w=64):
    out = np.empty((b, C, T), np.float32)
    for core in range(8):
        s, half = divmod(core, 2)
        out[s][:, half * NQ:(half + 1) * NQ] = results[core]["y"]
    return out.reshape(b, C, h, w)


_NC = None


def get_nc():
    global _NC
    if _NC is None:
        _NC = build_bass()
    return _NC


def kernel(**inputs):
    in_maps = make_in_maps(**inputs)
    nc = get_nc()
    res = run_bass_kernel_spmd(nc, in_maps, core_ids=list(range(8)))
    return assemble_output(res.results)


if __name__ == "__main__":
    nc = get_nc()
    print("built + compiled ok")


# revision 8
# speedup vs baseline: 24.0155x; 1.0268x over previous
"""Attention2D Trainium2 Bass kernel.

Reference computation (per sample s of 4):
    x  = GroupNorm32(q[s])                      # [512, 4096] (c, hw)
    qp = Wq xn + bq ; kp = Wk xn + bk ; vp = Wv xn + bv
    S[i, j]  = sum_c kp[c, i] qp[c, j] / sqrt(512)
    A[:, j]  = softmax_i(S[:, j])
    out[c,j] = sum_i vp[c, i] A[i, j]
    y        = (Wo out + bo + q[s]) / sqrt(2)

Sharding: 8 cores = 4 samples x 2 query-halves (2048 tokens each).
The host permutes the token axis per core so the core's query half is
always tokens [0:2048) -> every core runs an identical program (SPMD,
no collectives).  Key/value work over all 4096 tokens is duplicated
between the two cores of a sample (cheap relative to attention).

On-chip layout: scores are computed as S[i(keys on partitions), j] so
that exp() is a single ScalarE pass PSUM->SBUF and the softmax
denominator Z[j] = sum_i E[i, j] is a ones-vector matmul on TensorE --
no transposes anywhere.  V is produced directly transposed (vfT[i, c])
by swapping matmul operands.  All matmul operands are bf16 (full PE
speed); accumulation in fp32 PSUM; GroupNorm stats in fp32.
"""

import numpy as np
import ml_dtypes

import concourse.bass as bass
import concourse.bacc as bacc
import concourse.tile as tile
import concourse.mybir as mybir
from concourse.bass_utils import run_bass_kernel_spmd

F32 = mybir.dt.float32
BF16 = mybir.dt.bfloat16
AF = mybir.ActivationFunctionType

P = 128          # partitions
C = 512          # channels
CT = C // P      # channel tiles (4)
T = 4096         # tokens per sample (h*w)
NQ = 2048        # query tokens per core
JC = 512         # query chunk (PSUM bank width in fp32)
NJ = NQ // JC    # query chunks per core (4)
IT = T // P      # key tiles (32)
NG_TILE = 8      # groups per channel tile (32 groups / 4 tiles)
GS = 16          # channels per group
EPS = 1e-6
SCALE = 1.0 / np.sqrt(C)
INV_SQRT2 = 0.7071067811865476


def _emit_body(nc, pools, aps):
    """One full forward pass. `pools` are long-lived tile pools; PSUM usage
    never exceeds 8 banks (pss 2 + psav 4 + psz 2)."""
    (const, big, sbx, st, sbe, sbw, sbq, sby, pss, psav, psz) = pools
    (x_d, xb_d, y_d, w_sb, vec_sb, bvrep, indf, indb, ones_bf, ones_f1,
     eps_t) = aps

    # ---- persistent activations ----
    xn = big.tile([P, CT, T], BF16, tag="xn")     # normalized input
    kf = big.tile([P, CT, T], BF16, tag="kf")     # K  [c, i]
    qf = big.tile([P, CT, NQ], BF16, tag="qf")    # Q  [c, j]
    vfT = big.tile([P, IT, C], BF16, tag="vfT")   # V^T [i, c]

    # ================= phase 1: GroupNorm =================
    # stats + normalization read the host-cast bf16 copy of x (half the HBM
    # traffic of f32; stats arithmetic stays f32)
    for t in range(CT):
        x_t = sbx.tile([P, T], BF16, tag="x")
        # DMA in halves so bn_stats can start on the first half early
        nc.sync.dma_start(
            out=x_t[:, 0:T // 2], in_=xb_d[t * P:(t + 1) * P, 0:T // 2])
        nc.sync.dma_start(
            out=x_t[:, T // 2:T], in_=xb_d[t * P:(t + 1) * P, T // 2:T])
        stats = st.tile([P, 8, 6], F32, tag="stats")
        for sg in range(8):
            nc.vector.bn_stats(
                out=stats[:, sg, :], in_=x_t[:, sg * 512:(sg + 1) * 512]
            )
        mv = st.tile([P, 2], F32, tag="mv")
        nc.vector.bn_aggr(out=mv, in_=stats)
        # t2 = [mean, E[x^2]] per channel
        t2 = st.tile([P, 2], F32, tag="t2")
        nc.vector.tensor_copy(out=t2[:, 0:1], in_=mv[:, 0:1])
        nc.vector.tensor_mul(t2[:, 1:2], mv[:, 0:1], mv[:, 0:1])
        nc.vector.tensor_add(t2[:, 1:2], t2[:, 1:2], mv[:, 1:2])
        # group-reduce (mean over the 16 channels of each group)
        g_ps = psz.tile([NG_TILE, 2], F32, tag="z")
        nc.tensor.matmul(g_ps, indf, t2, start=True, stop=True)
        gm = st.tile([NG_TILE, 2], F32, tag="gm")
        nc.vector.tensor_copy(out=gm, in_=g_ps)
        var = st.tile([NG_TILE, 1], F32, tag="var")
        nc.vector.tensor_mul(var, gm[:, 0:1], gm[:, 0:1])
        nc.vector.tensor_sub(var, gm[:, 1:2], var)
        sd = st.tile([NG_TILE, 1], F32, tag="sd")
        nc.scalar.activation(
            out=sd, in_=var, func=AF.Sqrt, bias=eps_t[0:NG_TILE, :]
        )
        rstd = st.tile([NG_TILE, 1], F32, tag="rstd")
        nc.vector.reciprocal(out=rstd, in_=sd)
        gm2 = st.tile([NG_TILE, 2], F32, tag="gm2")
        nc.vector.tensor_copy(out=gm2[:, 0:1], in_=gm[:, 0:1])
        nc.vector.tensor_copy(out=gm2[:, 1:2], in_=rstd)
        # broadcast group stats back to channels
        bc_ps = psz.tile([P, 2], F32, tag="z")
        nc.tensor.matmul(bc_ps, indb, gm2, start=True, stop=True)
        a_t = st.tile([P, 1], F32, tag="a")
        b_t = st.tile([P, 1], F32, tag="b")
        nc.vector.tensor_mul(a_t, bc_ps[:, 1:2], vec_sb["gamma"][:, t:t + 1])
        nc.vector.tensor_mul(b_t, bc_ps[:, 0:1], a_t)
        nc.vector.tensor_sub(b_t, vec_sb["beta"][:, t:t + 1], b_t)
        # normalize in halves (finer pipelining into the QKV phase)
        for h in range(2):
            nc.scalar.activation(
                out=xn[:, t, h * (T // 2):(h + 1) * (T // 2)],
                in_=x_t[:, h * (T // 2):(h + 1) * (T // 2)],
                func=AF.Identity, bias=b_t, scale=a_t,
            )

    # ================= phase 2: Q/K/V projections =================
    # PSUM: alternate between the two pools -> 6 effective buffers.
    def proj_psum(idx, shape):
        pool, tag = ((pss, "s"), (psav, "av"))[idx % 2]
        return pool.tile(shape, F32, tag=tag, name=f"pp{idx % 2}")

    # K: kf[c_out, i] ; Q: qf[c_out, j] (queries are tokens 0:NQ)
    pidx = 0
    for t_out in range(CT):
        for ic in range(T // JC):
            kp = proj_psum(pidx, [P, JC]); pidx += 1
            for t in range(CT):
                nc.tensor.matmul(
                    kp,
                    w_sb["wkT"][:, t, t_out * P:(t_out + 1) * P],
                    xn[:, t, ic * JC:(ic + 1) * JC],
                    start=(t == 0), stop=(t == CT - 1),
                )
            nc.vector.tensor_scalar_add(
                out=kf[:, t_out, ic * JC:(ic + 1) * JC], in0=kp,
                scalar1=vec_sb["bk"][:, t_out:t_out + 1],
            )
    for t_out in range(CT):
        for jc in range(NJ):
            qp = proj_psum(pidx, [P, JC]); pidx += 1
            for t in range(CT):
                nc.tensor.matmul(
                    qp,
                    w_sb["wqT"][:, t, t_out * P:(t_out + 1) * P],
                    xn[:, t, jc * JC:(jc + 1) * JC],
                    start=(t == 0), stop=(t == CT - 1),
                )
            nc.vector.tensor_scalar_add(
                out=qf[:, t_out, jc * JC:(jc + 1) * JC], in0=qp,
                scalar1=vec_sb["bq"][:, t_out:t_out + 1],
            )
    # V, produced transposed: vfT[i, c] = sum_c' xn[c', i] wvT[c', c]
    for k in range(IT):
        vp = proj_psum(pidx, [P, C]); pidx += 1
        for t in range(CT):
            nc.tensor.matmul(
                vp,
                xn[:, t, k * P:(k + 1) * P],
                w_sb["wvT"][:, t, :],
                start=(t == 0), stop=(t == CT - 1),
            )
        nc.vector.tensor_add(vfT[:, k, :], vp, bvrep)

    # ================= phase 3: attention =================
    for jc in range(NJ):
        # residual input for this chunk (original x, queries 0:NQ)
        xqs = []
        for mo in range(CT):
            xq_t = sbq.tile([P, JC], F32, tag="xq")
            nc.sync.dma_start(
                out=xq_t,
                in_=x_d[mo * P:(mo + 1) * P, jc * JC:(jc + 1) * JC],
            )
            xs = sbq.tile([P, JC], F32, tag="xqs")
            # on DVE so phase-3 ScalarE stays pure-Exp (no LUT reloads)
            nc.vector.tensor_scalar(
                out=xs, in0=xq_t,
                scalar1=INV_SQRT2, scalar2=vec_sb["bo2"][:, mo:mo + 1],
                op0=mybir.AluOpType.mult, op1=mybir.AluOpType.add,
            )
            xqs.append(xs)

        z_ps = psz.tile([1, JC], F32, tag="z")
        av_ps = [psav.tile([P, JC], F32, tag="av", name=f"av{m}") for m in range(CT)]

        def av_step(k, e_t):
            nc.tensor.matmul(
                z_ps, ones_bf, e_t, start=(k == 0), stop=(k == IT - 1)
            )
            for m in range(CT):
                nc.tensor.matmul(
                    av_ps[m],
                    vfT[:, k, m * P:(m + 1) * P],
                    e_t,
                    start=(k == 0), stop=(k == IT - 1),
                )

        prev_e = None
        for k in range(IT):
            s_ps = pss.tile([P, JC], F32, tag="s")
            for t in range(CT):
                nc.tensor.matmul(
                    s_ps,
                    kf[:, t, k * P:(k + 1) * P],
                    qf[:, t, jc * JC:(jc + 1) * JC],
                    start=(t == 0), stop=(t == CT - 1),
                )
            e_t = sbe.tile([P, JC], BF16, tag="e")
            nc.scalar.activation(out=e_t, in_=s_ps, func=AF.Exp, scale=SCALE)
            if prev_e is not None:
                av_step(k - 1, prev_e)
            prev_e = e_t
        av_step(IT - 1, prev_e)
        # normalize: out_n = av / Z
        zinv = sbw.tile([1, JC], F32, tag="zinv")
        nc.vector.reciprocal(out=zinv, in_=z_ps)
        zbc_ps = psz.tile([P, JC], F32, tag="z")
        nc.tensor.matmul(zbc_ps, ones_f1, zinv, start=True, stop=True)
        zrep = sbw.tile([P, JC], F32, tag="zrep")
        nc.vector.tensor_copy(out=zrep, in_=zbc_ps)
        out_n = sbw.tile([P, CT, JC], BF16, tag="outn")
        for m in range(CT):
            nc.vector.tensor_mul(out_n[:, m, :], av_ps[m], zrep)
        # output projection + residual
        for mo in range(CT):
            y_ps = psav.tile([P, JC], F32, tag="av")
            for m in range(CT):
                nc.tensor.matmul(
                    y_ps,
                    w_sb["woT"][:, m, mo * P:(mo + 1) * P],
                    out_n[:, m, :],
                    start=(m == 0), stop=(m == CT - 1),
                )
            yf = sby.tile([P, JC], F32, tag="y")
            nc.vector.scalar_tensor_tensor(
                out=yf, in0=y_ps, scalar=INV_SQRT2, in1=xqs[mo],
                op0=mybir.AluOpType.mult, op1=mybir.AluOpType.add,
            )
            nc.sync.dma_start(
                out=y_d[mo * P:(mo + 1) * P, jc * JC:(jc + 1) * JC],
                in_=yf,
            )


def build_bass(loop_iters=None):
    """loop_iters=None: single-shot kernel.  loop_iters=R: wrap the body in a
    hardware For_i loop (for on-device timing; everything re-runs each
    iteration, output is idempotent)."""
    nc = bacc.Bacc("TRN2", target_bir_lowering=False, debug=False)

    x_d = nc.dram_tensor("x", [C, T], F32, kind="ExternalInput").ap()
    xb_d = nc.dram_tensor("xb", [C, T], BF16, kind="ExternalInput").ap()
    w_d = {
        n: nc.dram_tensor(n, [C, C], BF16, kind="ExternalInput").ap()
        for n in ("wqT", "wkT", "wvT", "woT")
    }
    # per-channel vectors in [p, tile] layout (c = t*128 + p)
    vec_d = {
        n: nc.dram_tensor(n, [P, CT], F32, kind="ExternalInput").ap()
        for n in ("bq", "bk", "bo2", "gamma", "beta")
    }
    bvrep_d = nc.dram_tensor("bv_rep", [P, C], F32, kind="ExternalInput").ap()
    indf_d = nc.dram_tensor("indf", [P, NG_TILE], F32, kind="ExternalInput").ap()
    indb_d = nc.dram_tensor("indb", [NG_TILE, P], F32, kind="ExternalInput").ap()
    onesb_d = nc.dram_tensor("ones_bf", [P, 1], BF16, kind="ExternalInput").ap()
    onesf_d = nc.dram_tensor("ones_f1", [1, P], F32, kind="ExternalInput").ap()
    y_d = nc.dram_tensor("y", [C, NQ], F32, kind="ExternalOutput").ap()

    with tile.TileContext(nc) as tc:
        with (
            tc.tile_pool(name="const", bufs=1) as const,
            tc.tile_pool(name="big", bufs=1) as big,
            tc.tile_pool(name="sbx", bufs=2) as sbx,
            tc.tile_pool(name="st", bufs=2) as st,
            tc.tile_pool(name="sbe", bufs=3) as sbe,
            tc.tile_pool(name="sbw", bufs=2) as sbw,
            tc.tile_pool(name="sbq", bufs=4) as sbq,
            tc.tile_pool(name="sby", bufs=4) as sby,
            tc.tile_pool(name="pss", bufs=2, space="PSUM") as pss,
            tc.tile_pool(name="psav", bufs=4, space="PSUM") as psav,
            tc.tile_pool(name="psz", bufs=2, space="PSUM") as psz,
        ):
            # ---- constants (loaded once, outside any timing loop) ----
            w_sb = {}
            for n, d in w_d.items():
                w_sb[n] = const.tile([P, CT, C], BF16, tag=n, name=n)
                nc.sync.dma_start(out=w_sb[n], in_=d.rearrange("(t p) o -> p t o", p=P))
            vec_sb = {}
            for n, d in vec_d.items():
                vec_sb[n] = const.tile([P, CT], F32, tag=n, name=n)
                nc.sync.dma_start(out=vec_sb[n], in_=d)
            bvrep = const.tile([P, C], F32, tag="bvrep")
            nc.sync.dma_start(out=bvrep, in_=bvrep_d)
            indf = const.tile([P, NG_TILE], F32, tag="indf")
            nc.sync.dma_start(out=indf, in_=indf_d)
            indb = const.tile([NG_TILE, P], F32, tag="indb")
            nc.sync.dma_start(out=indb, in_=indb_d)
            ones_bf = const.tile([P, 1], BF16, tag="onesb")
            nc.sync.dma_start(out=ones_bf, in_=onesb_d)
            ones_f1 = const.tile([1, P], F32, tag="onesf")
            nc.sync.dma_start(out=ones_f1, in_=onesf_d)
            eps_t = const.tile([P, 1], F32, tag="eps")
            nc.vector.memset(eps_t, EPS)

            pools = (const, big, sbx, st, sbe, sbw, sbq, sby, pss, psav, psz)
            aps = (x_d, xb_d, y_d, w_sb, vec_sb, bvrep, indf, indb,
                   ones_bf, ones_f1, eps_t)
            if loop_iters is None:
                _emit_body(nc, pools, aps)
            else:
                with tc.For_i(0, loop_iters, 1):
                    _emit_body(nc, pools, aps)
    nc.compile()
    return nc


def make_in_maps(q, gamma, beta, wq, bq, wk, bk, wv, bv, wo, bo):
    """Host-side prep: per-core permuted x + replicated (pre-transposed) weights."""
    f32 = np.float32
    bf16 = ml_dtypes.bfloat16
    q = np.asarray(q, f32)
    b = q.shape[0]
    x = q.reshape(b, C, T)

    def pt(v):  # [512] -> [128, 4] (c = t*128 + p)
        return np.ascontiguousarray(np.asarray(v, f32).reshape(CT, P).T)

    common = {
        "wqT": np.ascontiguousarray(np.asarray(wq, f32).T).astype(bf16),
        "wkT": np.ascontiguousarray(np.asarray(wk, f32).T).astype(bf16),
        "wvT": np.ascontiguousarray(np.asarray(wv, f32).T).astype(bf16),
        "woT": np.ascontiguousarray(np.asarray(wo, f32).T).astype(bf16),
        "bq": pt(bq), "bk": pt(bk), "bo2": pt(np.asarray(bo, f32) * INV_SQRT2),
        "gamma": pt(gamma), "beta": pt(beta),
        "bv_rep": np.ascontiguousarray(
            np.broadcast_to(np.asarray(bv, f32), (P, C))),
        "indf": np.ascontiguousarray(
            (np.arange(P)[:, None] // GS == np.arange(NG_TILE)[None, :])
            .astype(f32) / GS),
        "indb": np.ascontiguousarray(
            (np.arange(P)[None, :] // GS == np.arange(NG_TILE)[:, None])
            .astype(f32)),
        "ones_bf": np.ones((P, 1), bf16),
        "ones_f1": np.ones((1, P), f32),
    }
    in_maps = []
    for core in range(8):
        s, half = divmod(core, 2)
        xs = x[s]
        if half == 0:
            xp = xs
        else:
            xp = np.concatenate([xs[:, NQ:], xs[:, :NQ]], axis=1)
        xpc = np.ascontiguousarray(xp)
        in_maps.append({"x": xpc, "xb": xpc.astype(bf16), **common})
    return in_maps


def assemble_output(results, b=4, h=64, w=64):
    out = np.empty((b, C, T), np.float32)
    for core in range(8):
        s, half = divmod(core, 2)
        out[s][:, half * NQ:(half + 1) * NQ] = results[core]["y"]
    return out.reshape(b, C, h, w)


_NC = None


def get_nc():
    global _NC
    if _NC is None:
        _NC = build_bass()
    return _NC


def kernel(**inputs):
    in_maps = make_in_maps(**inputs)
    nc = get_nc()
    res = run_bass_kernel_spmd(nc, in_maps, core_ids=list(range(8)))
    return assemble_output(res.results)


if __name__ == "__main__":
    nc = get_nc()
    print("built + compiled ok")


# revision 10
# speedup vs baseline: 26.5494x; 1.1055x over previous
"""Attention2D Trainium2 Bass kernel.

Reference computation (per sample s of 4):
    x  = GroupNorm32(q[s])                      # [512, 4096] (c, hw)
    qp = Wq xn + bq ; kp = Wk xn + bk ; vp = Wv xn + bv
    S[i, j]  = sum_c kp[c, i] qp[c, j] / sqrt(512)
    A[:, j]  = softmax_i(S[:, j])
    out[c,j] = sum_i vp[c, i] A[i, j]
    y        = (Wo out + bo + q[s]) / sqrt(2)

Sharding: 8 cores = 4 samples x 2 query-halves (2048 tokens each).
The host permutes the token axis per core so the core's query half is
always tokens [0:2048) -> every core runs an identical program (SPMD,
no collectives).  Key/value work over all 4096 tokens is duplicated
between the two cores of a sample (cheap relative to attention).

On-chip layout: scores are computed as S[i(keys on partitions), j] so
that exp() is a single ScalarE pass PSUM->SBUF and the softmax
denominator Z[j] = sum_i E[i, j] is a ones-vector matmul on TensorE --
no transposes anywhere.  V is produced directly transposed (vfT[i, c])
by swapping matmul operands.  All matmul operands are bf16 (full PE
speed); accumulation in fp32 PSUM; GroupNorm stats in fp32.
"""

import numpy as np
import ml_dtypes

import concourse.bass as bass
import concourse.bacc as bacc
import concourse.tile as tile
import concourse.mybir as mybir
from concourse.bass_utils import run_bass_kernel_spmd

F32 = mybir.dt.float32
BF16 = mybir.dt.bfloat16
AF = mybir.ActivationFunctionType

P = 128          # partitions
C = 512          # channels
CT = C // P      # channel tiles (4)
T = 4096         # tokens per sample (h*w)
NQ = 2048        # query tokens per core
JC = 512         # query chunk (PSUM bank width in fp32)
NJ = NQ // JC    # query chunks per core (4)
IT = T // P      # key tiles (32)
NG_TILE = 8      # groups per channel tile (32 groups / 4 tiles)
GS = 16          # channels per group
EPS = 1e-6
SCALE = 1.0 / np.sqrt(C)
INV_SQRT2 = 0.7071067811865476


def _emit_body(nc, pools, aps, nj=NJ, do_attn=True):
    """One full forward pass. `pools` are long-lived tile pools; PSUM usage
    never exceeds 8 banks (pss 2 + psav 4 + psz 2)."""
    (const, big, sbx, st, sbe, sbw, sbq, sby, pss, psav, psz) = pools
    (x_d, xb_d, y_d, w_sb, vec_sb, bvrep, indf, indb, ones_bf, ones_f1,
     ones_fc, eps_t) = aps

    # ---- persistent activations ----
    xn = big.tile([P, CT, T], BF16, tag="xn")     # normalized input
    kf = big.tile([P, CT, T], BF16, tag="kf")     # K  [c, i]
    qf = big.tile([P, CT, NQ], BF16, tag="qf")    # Q  [c, j]
    vfT = big.tile([P, IT, C], BF16, tag="vfT")   # V^T [i, c]

    # ================= phase 1: GroupNorm =================
    # stats + normalization read the host-cast bf16 copy of x (half the HBM
    # traffic of f32; stats arithmetic stays f32)
    for t in range(CT):
        x_t = sbx.tile([P, T], BF16, tag="x")
        # DMA in halves so bn_stats can start on the first half early
        nc.sync.dma_start(
            out=x_t[:, 0:T // 2], in_=xb_d[t * P:(t + 1) * P, 0:T // 2])
        nc.sync.dma_start(
            out=x_t[:, T // 2:T], in_=xb_d[t * P:(t + 1) * P, T // 2:T])
        stats = st.tile([P, 8, 6], F32, tag="stats")
        for sg in range(8):
            nc.vector.bn_stats(
                out=stats[:, sg, :], in_=x_t[:, sg * 512:(sg + 1) * 512]
            )
        mv = st.tile([P, 2], F32, tag="mv")
        nc.vector.bn_aggr(out=mv, in_=stats)
        # t2 = [mean, E[x^2]] per channel
        t2 = st.tile([P, 2], F32, tag="t2")
        nc.vector.tensor_copy(out=t2[:, 0:1], in_=mv[:, 0:1])
        nc.vector.tensor_mul(t2[:, 1:2], mv[:, 0:1], mv[:, 0:1])
        nc.vector.tensor_add(t2[:, 1:2], t2[:, 1:2], mv[:, 1:2])
        # group-reduce (mean over the 16 channels of each group)
        g_ps = psz.tile([NG_TILE, 2], F32, tag="z")
        nc.tensor.matmul(g_ps, indf, t2, start=True, stop=True)
        gm = st.tile([NG_TILE, 2], F32, tag="gm")
        nc.vector.tensor_copy(out=gm, in_=g_ps)
        var = st.tile([NG_TILE, 1], F32, tag="var")
        nc.vector.tensor_mul(var, gm[:, 0:1], gm[:, 0:1])
        nc.vector.tensor_sub(var, gm[:, 1:2], var)
        sd = st.tile([NG_TILE, 1], F32, tag="sd")
        nc.scalar.activation(
            out=sd, in_=var, func=AF.Sqrt, bias=eps_t[0:NG_TILE, :]
        )
        rstd = st.tile([NG_TILE, 1], F32, tag="rstd")
        nc.vector.reciprocal(out=rstd, in_=sd)
        gm2 = st.tile([NG_TILE, 2], F32, tag="gm2")
        nc.vector.tensor_copy(out=gm2[:, 0:1], in_=gm[:, 0:1])
        nc.vector.tensor_copy(out=gm2[:, 1:2], in_=rstd)
        # broadcast group stats back to channels
        bc_ps = psz.tile([P, 2], F32, tag="z")
        nc.tensor.matmul(bc_ps, indb, gm2, start=True, stop=True)
        a_t = st.tile([P, 1], F32, tag="a")
        b_t = st.tile([P, 1], F32, tag="b")
        nc.vector.tensor_mul(a_t, bc_ps[:, 1:2], vec_sb["gamma"][:, t:t + 1])
        nc.vector.tensor_mul(b_t, bc_ps[:, 0:1], a_t)
        nc.vector.tensor_sub(b_t, vec_sb["beta"][:, t:t + 1], b_t)
        # normalize in halves (finer pipelining into the QKV phase)
        for h in range(2):
            nc.scalar.activation(
                out=xn[:, t, h * (T // 2):(h + 1) * (T // 2)],
                in_=x_t[:, h * (T // 2):(h + 1) * (T // 2)],
                func=AF.Identity, bias=b_t, scale=a_t,
            )

    # ================= phase 2: Q/K/V projections =================
    # PSUM: alternate between the two pools -> 6 effective buffers.
    def proj_psum(idx, shape):
        pool, tag = ((pss, "s"), (psav, "av"))[idx % 2]
        return pool.tile(shape, F32, tag=tag, name=f"pp{idx % 2}")

    # K: kf[c_out, i] ; Q: qf[c_out, j] (queries are tokens 0:NQ)
    pidx = 0
    for t_out in range(CT):
        for ic in range(T // JC):
            kp = proj_psum(pidx, [P, JC]); pidx += 1
            for t in range(CT):
                nc.tensor.matmul(
                    kp,
                    w_sb["wkT"][:, t, t_out * P:(t_out + 1) * P],
                    xn[:, t, ic * JC:(ic + 1) * JC],
                    start=(t == 0), stop=(t == CT - 1),
                )
            nc.scalar.activation(
                out=kf[:, t_out, ic * JC:(ic + 1) * JC], in_=kp,
                func=AF.Identity, bias=vec_sb["bk"][:, t_out:t_out + 1],
            )
    for t_out in range(CT):
        for jc in range(NJ):
            qp = proj_psum(pidx, [P, JC]); pidx += 1
            for t in range(CT):
                nc.tensor.matmul(
                    qp,
                    w_sb["wqT"][:, t, t_out * P:(t_out + 1) * P],
                    xn[:, t, jc * JC:(jc + 1) * JC],
                    start=(t == 0), stop=(t == CT - 1),
                )
            nc.scalar.activation(
                out=qf[:, t_out, jc * JC:(jc + 1) * JC], in_=qp,
                func=AF.Identity, bias=vec_sb["bq"][:, t_out:t_out + 1],
            )
    # V, produced transposed: vfT[i, c] = sum_c' xn[c', i] wvT[c', c]
    for k in range(IT):
        vp = proj_psum(pidx, [P, C]); pidx += 1
        for t in range(CT):
            nc.tensor.matmul(
                vp,
                xn[:, t, k * P:(k + 1) * P],
                w_sb["wvT"][:, t, :],
                start=(t == 0), stop=(t == CT - 1),
            )
        nc.vector.tensor_add(vfT[:, k, :], vp, bvrep)

    # ================= phase 3: attention =================
    if not do_attn:
        return
    for jc in range(nj):
        # residual input for this chunk (original x, queries 0:NQ)
        xqs = []
        for mo in range(CT):
            xq_t = sbq.tile([P, JC], F32, tag="xq")
            nc.sync.dma_start(
                out=xq_t,
                in_=x_d[mo * P:(mo + 1) * P, jc * JC:(jc + 1) * JC],
            )
            xs = sbq.tile([P, JC], F32, tag="xqs")
            # on DVE so phase-3 ScalarE stays pure-Exp (no LUT reloads)
            nc.vector.tensor_scalar(
                out=xs, in0=xq_t,
                scalar1=INV_SQRT2, scalar2=vec_sb["bo2"][:, mo:mo + 1],
                op0=mybir.AluOpType.mult, op1=mybir.AluOpType.add,
            )
            xqs.append(xs)

        zacc = sbw.tile([P, JC], F32, tag="zacc")
        av_ps = [psav.tile([P, JC], F32, tag="av", name=f"av{m}") for m in range(CT)]

        def av_step(k, e_t):
            if k == 0:
                nc.vector.tensor_copy(out=zacc, in_=e_t)
            else:
                nc.vector.tensor_add(zacc, zacc, e_t)
            for m in range(CT):
                nc.tensor.matmul(
                    av_ps[m],
                    vfT[:, k, m * P:(m + 1) * P],
                    e_t,
                    start=(k == 0), stop=(k == IT - 1),
                )

        prev_e = None
        for k in range(IT):
            s_ps = pss.tile([P, JC], F32, tag="s")
            for t in range(CT):
                nc.tensor.matmul(
                    s_ps,
                    kf[:, t, k * P:(k + 1) * P],
                    qf[:, t, jc * JC:(jc + 1) * JC],
                    start=(t == 0), stop=(t == CT - 1),
                )
            e_t = sbe.tile([P, JC], BF16, tag="e")
            nc.scalar.activation(out=e_t, in_=s_ps, func=AF.Exp, scale=SCALE)
            if prev_e is not None:
                av_step(k - 1, prev_e)
            prev_e = e_t
        av_step(IT - 1, prev_e)
        # normalize: out_n = av / Z  (Z = cross-partition sum of zacc via PE)
        z_ps = psz.tile([1, JC], F32, tag="z")
        nc.tensor.matmul(z_ps, ones_fc, zacc, start=True, stop=True)
        zinv = sbw.tile([1, JC], F32, tag="zinv")
        nc.vector.reciprocal(out=zinv, in_=z_ps)
        zbc_ps = psz.tile([P, JC], F32, tag="z")
        nc.tensor.matmul(zbc_ps, ones_f1, zinv, start=True, stop=True)
        zrep = sbw.tile([P, JC], F32, tag="zrep")
        nc.vector.tensor_copy(out=zrep, in_=zbc_ps)
        out_n = sbw.tile([P, CT, JC], BF16, tag="outn")
        for m in range(CT):
            nc.vector.tensor_mul(out_n[:, m, :], av_ps[m], zrep)
        # output projection + residual (PSUM from psz so next chunk's AV
        # accumulators don't contend with this chunk's tail)
        for mo in range(CT):
            y_ps = psz.tile([P, JC], F32, tag="z")
            for m in range(CT):
                nc.tensor.matmul(
                    y_ps,
                    w_sb["woT"][:, m, mo * P:(mo + 1) * P],
                    out_n[:, m, :],
                    start=(m == 0), stop=(m == CT - 1),
                )
            yf = sby.tile([P, JC], F32, tag="y")
            nc.vector.scalar_tensor_tensor(
                out=yf, in0=y_ps, scalar=INV_SQRT2, in1=xqs[mo],
                op0=mybir.AluOpType.mult, op1=mybir.AluOpType.add,
            )
            nc.sync.dma_start(
                out=y_d[mo * P:(mo + 1) * P, jc * JC:(jc + 1) * JC],
                in_=yf,
            )


def build_bass(loop_iters=None, nj=NJ, do_attn=True):
    """loop_iters=None: single-shot kernel.  loop_iters=R: wrap the body in a
    hardware For_i loop (for on-device timing; everything re-runs each
    iteration, output is idempotent)."""
    nc = bacc.Bacc("TRN2", target_bir_lowering=False, debug=False)

    x_d = nc.dram_tensor("x", [C, T], F32, kind="ExternalInput").ap()
    xb_d = nc.dram_tensor("xb", [C, T], BF16, kind="ExternalInput").ap()
    w_d = {
        n: nc.dram_tensor(n, [C, C], BF16, kind="ExternalInput").ap()
        for n in ("wqT", "wkT", "wvT", "woT")
    }
    # per-channel vectors in [p, tile] layout (c = t*128 + p)
    vec_d = {
        n: nc.dram_tensor(n, [P, CT], F32, kind="ExternalInput").ap()
        for n in ("bq", "bk", "bo2", "gamma", "beta")
    }
    bvrep_d = nc.dram_tensor("bv_rep", [P, C], F32, kind="ExternalInput").ap()
    indf_d = nc.dram_tensor("indf", [P, NG_TILE], F32, kind="ExternalInput").ap()
    indb_d = nc.dram_tensor("indb", [NG_TILE, P], F32, kind="ExternalInput").ap()
    onesb_d = nc.dram_tensor("ones_bf", [P, 1], BF16, kind="ExternalInput").ap()
    onesc_d = nc.dram_tensor("ones_fc", [P, 1], F32, kind="ExternalInput").ap()
    onesf_d = nc.dram_tensor("ones_f1", [1, P], F32, kind="ExternalInput").ap()
    y_d = nc.dram_tensor("y", [C, NQ], F32, kind="ExternalOutput").ap()

    with tile.TileContext(nc) as tc:
        with (
            tc.tile_pool(name="const", bufs=1) as const,
            tc.tile_pool(name="big", bufs=1) as big,
            tc.tile_pool(name="sbx", bufs=2) as sbx,
            tc.tile_pool(name="st", bufs=2) as st,
            tc.tile_pool(name="sbe", bufs=3) as sbe,
            tc.tile_pool(name="sbw", bufs=2) as sbw,
            tc.tile_pool(name="sbq", bufs=4) as sbq,
            tc.tile_pool(name="sby", bufs=4) as sby,
            tc.tile_pool(name="pss", bufs=2, space="PSUM") as pss,
            tc.tile_pool(name="psav", bufs=4, space="PSUM") as psav,
            tc.tile_pool(name="psz", bufs=2, space="PSUM") as psz,
        ):
            # ---- constants (loaded once, outside any timing loop) ----
            w_sb = {}
            for n, d in w_d.items():
                w_sb[n] = const.tile([P, CT, C], BF16, tag=n, name=n)
                nc.sync.dma_start(out=w_sb[n], in_=d.rearrange("(t p) o -> p t o", p=P))
            vec_sb = {}
            for n, d in vec_d.items():
                vec_sb[n] = const.tile([P, CT], F32, tag=n, name=n)
                nc.sync.dma_start(out=vec_sb[n], in_=d)
            bvrep = const.tile([P, C], F32, tag="bvrep")
            nc.sync.dma_start(out=bvrep, in_=bvrep_d)
            indf = const.tile([P, NG_TILE], F32, tag="indf")
            nc.sync.dma_start(out=indf, in_=indf_d)
            indb = const.tile([NG_TILE, P], F32, tag="indb")
            nc.sync.dma_start(out=indb, in_=indb_d)
            ones_bf = const.tile([P, 1], BF16, tag="onesb")
            nc.sync.dma_start(out=ones_bf, in_=onesb_d)
            ones_f1 = const.tile([1, P], F32, tag="onesf")
            nc.sync.dma_start(out=ones_f1, in_=onesf_d)
            ones_fc = const.tile([P, 1], F32, tag="onesc")
            nc.sync.dma_start(out=ones_fc, in_=onesc_d)
            eps_t = const.tile([P, 1], F32, tag="eps")
            nc.vector.memset(eps_t, EPS)

            pools = (const, big, sbx, st, sbe, sbw, sbq, sby, pss, psav, psz)
            aps = (x_d, xb_d, y_d, w_sb, vec_sb, bvrep, indf, indb,
                   ones_bf, ones_f1, ones_fc, eps_t)
            if loop_iters is None:
                _emit_body(nc, pools, aps, nj=nj, do_attn=do_attn)
            else:
                with tc.For_i(0, loop_iters, 1):
                    _emit_body(nc, pools, aps, nj=nj, do_attn=do_attn)
    nc.compile()
    return nc


def make_in_maps(q, gamma, beta, wq, bq, wk, bk, wv, bv, wo, bo):
    """Host-side prep: per-core permuted x + replicated (pre-transposed) weights."""
    f32 = np.float32
    bf16 = ml_dtypes.bfloat16
    q = np.asarray(q, f32)
    b = q.shape[0]
    x = q.reshape(b, C, T)

    def pt(v):  # [512] -> [128, 4] (c = t*128 + p)
        return np.ascontiguousarray(np.asarray(v, f32).reshape(CT, P).T)

    common = {
        "wqT": np.ascontiguousarray(np.asarray(wq, f32).T).astype(bf16),
        "wkT": np.ascontiguousarray(np.asarray(wk, f32).T).astype(bf16),
        "wvT": np.ascontiguousarray(np.asarray(wv, f32).T).astype(bf16),
        "woT": np.ascontiguousarray(np.asarray(wo, f32).T).astype(bf16),
        "bq": pt(bq), "bk": pt(bk), "bo2": pt(np.asarray(bo, f32) * INV_SQRT2),
        "gamma": pt(gamma), "beta": pt(beta),
        "bv_rep": np.ascontiguousarray(
            np.broadcast_to(np.asarray(bv, f32), (P, C))),
        "indf": np.ascontiguousarray(
            (np.arange(P)[:, None] // GS == np.arange(NG_TILE)[None, :])
            .astype(f32) / GS),
        "indb": np.ascontiguousarray(
            (np.arange(P)[None, :] // GS == np.arange(NG_TILE)[:, None])
            .astype(f32)),
        "ones_bf": np.ones((P, 1), bf16),
        "ones_fc": np.ones((P, 1), f32),
        "ones_f1": np.ones((1, P), f32),
    }
    in_maps = []
    for core in range(8):
        s, half = divmod(core, 2)
        xs = x[s]
        if half == 0:
            xp = xs
        else:
            xp = np.concatenate([xs[:, NQ:], xs[:, :NQ]], axis=1)
        xpc = np.ascontiguousarray(xp)
        in_maps.append({"x": xpc, "xb": xpc.astype(bf16), **common})
    return in_maps


def assemble_output(results, b=4, h=64, w=64):
    out = np.empty((b, C, T), np.float32)
    for core in range(8):
        s, half = divmod(core, 2)
        out[s][:, half * NQ:(half + 1) * NQ] = results[core]["y"]
    return out.reshape(b, C, h, w)


_NC = None


def get_nc():
    global _NC
    if _NC is None:
        _NC = build_bass()
    return _NC


def kernel(**inputs):
    in_maps = make_in_maps(**inputs)
    nc = get_nc()
    res = run_bass_kernel_spmd(nc, in_maps, core_ids=list(range(8)))
    return assemble_output(res.results)


if __name__ == "__main__":
    nc = get_nc()
    print("built + compiled ok")
